# revision 27
# baseline (speedup 1.0000x reference)
"""GAT head (DGAT) Trainium2 kernel: 8-core row-sharded masked-softmax attention.

Math (per reference):
  h = X @ W                       [N, 64]
  s_ij = src_i + dst_j, src = h@a[:64], dst = h@a[64:]
  att = softmax(where(adj>0, leaky_relu(s, 0.2), -inf), axis=1)
  out = elu(att @ h)

Key factorization: with E = exp(s) = exp(src_i)*exp(dst_j),
  exp(leaky_relu(s)) = max(E, E^0.2)
so the masked unnormalized attention is
  p[j,i] = max(adjT[j,i]*esrc_i*edst_j, adjT[j,i]*esrc02_i*edst02_j)
         = edst02_j * max(u1[j,i]*r_j, u2[j,i]),  r_j = exp(0.8*dst_j)
The i-dependent factors ride the (mandatory) adj transpose matmul as
diagonal rhs:  u1 = adjT @ diag(esrc), u2 = adjT @ diag(esrc02). The
leading edst02_j factor folds into the PRECOMPUTED hext (hext2[j,f] =
hext[j,f]*edst02_j, denominator row 64 = edst02_j), so stage B needs
only ONE elementwise op per p element:
  ACT: t1 = u1 * r_j  (scaled PSUM->SBUF copy; PSUM limits stt to one
       PSUM operand, so the copy hop stays)
  DVE: p' = max(u2, t1)  (tensor_tensor max)
and agg = hext2^T @ p' equals the original hext^T @ p exactly.
Per j-chunk engine busy: PE ~1.28us, DVE ~1.3us, ACT 0 (eliminated).

DMA split: adj is packed to uint8 on the host and streamed via Pool
SWDGE with a cast to bf16 {0,1.0};
SP (HWDGE) carries X fp32, consumed via float32r matmuls (1 cyc/col).
A dedicated early-src path (dynamic-offset DMA of this core's X columns)
gets the diag(exp(src)) tiles ready ~8us in so stage B starts early.

Aggregation: agg[65, i] += [h | 1]^T @ p  (row 64 = softmax denominator),
finalize: transpose agg, scale by 1/denom, ELU, DMA out.
"""
import os
import sys
import numpy as np

sys.path.insert(0, "/opt/trn_rl_repo")

import concourse.bass as bass
import concourse.bacc as bacc
import concourse.tile as tile
from concourse import mybir
from concourse.masks import make_identity
from concourse import bass_utils

P = 128
N = 8192
DIN = 256
DOUT = 64
NCORES = 8
R = N // NCORES          # rows per core
JT_W = 2048              # j supertile width
NJT = N // JT_W
JCPJT = JT_W // P
NJC = N // P             # 64 j-chunks
SUBS = R // P            # 8 row sub-blocks per core
F32 = mybir.dt.float32
F32R = mybir.dt.float32r
BF16 = mybir.dt.bfloat16
I32 = mybir.dt.int32
U8 = mybir.dt.uint8
FP8 = mybir.dt.float8e4

_cached = {}
ADJ_BUFS = 24
UBUFS = 3
ABLATE_DMA = bool(int(os.environ.get("ABLATE_DMA", "0")))   # dev-only timing probe
ABLATE_DVE = bool(int(os.environ.get("ABLATE_DVE", "0")))   # dev-only timing probe
ABLATE_ACT = bool(int(os.environ.get("ABLATE_ACT", "0")))   # dev-only timing probe


def build_module(rep=1, rep_loop=1):
    key = ("nc", rep, rep_loop)
    if key in _cached:
        return _cached[key]
    nc = bacc.Bacc("TRN2", target_bir_lowering=False, debug=False, num_devices=NCORES)

    adj_d = nc.dram_tensor("adjslab", [R, N], U8, kind="ExternalInput").ap()
    x_d = nc.dram_tensor("xt", [DIN, N], F32, kind="ExternalInput").ap()
    w_d = nc.dram_tensor("w", [DIN, DOUT], F32, kind="ExternalInput").ap()
    a_d = nc.dram_tensor("av", [2 * DOUT, 1], F32, kind="ExternalInput").ap()
    out_d = nc.dram_tensor("out", [R, DOUT], F32, kind="ExternalOutput").ap()

    with tile.TileContext(nc) as tc:
        for _ in range(rep):
            _build(nc, tc, adj_d, x_d, w_d, a_d, out_d, rep_loop)

    nc.compile()
    _cached[key] = nc
    return nc


def _build(nc, tc, adj_d, x_d, w_d, a_d, out_d, rep_loop=1):
    from contextlib import ExitStack, nullcontext

    with ExitStack() as ctx:
        const = ctx.enter_context(tc.tile_pool(name="const", bufs=1))

        # ---- constants ----
        idbf = const.tile([P, P], BF16)
        make_identity(nc, idbf)
        idf = const.tile([P, P], F32)
        make_identity(nc, idf)

        w_f = const.tile([P, 2 * DOUT], F32)   # [din_blk, (a|b) x 64]
        nc.sync.dma_start(out=w_f[:, 0:DOUT], in_=w_d[0:P, :])
        nc.sync.dma_start(out=w_f[:, DOUT : 2 * DOUT], in_=w_d[P:DIN, :])
        w_fr = const.tile([P, 2 * DOUT], F32R)  # f32r twin for PE use
        nc.sync.dma_start(out=w_fr[:, 0:DOUT], in_=w_d[0:P, :].bitcast(F32R))
        nc.sync.dma_start(out=w_fr[:, DOUT : 2 * DOUT], in_=w_d[P:DIN, :].bitcast(F32R))
        a1_f = const.tile([DOUT, 1], F32)
        a2_f = const.tile([DOUT, 1], F32)
        nc.sync.dma_start(out=a1_f, in_=a_d[0:DOUT, :])
        nc.sync.dma_start(out=a2_f, in_=a_d[DOUT : 2 * DOUT, :])

        # persistent per-core data
        hext_all = const.tile([P, NJC * 65], BF16)   # hext2: [j%P, 64 h*edst02 cols + edst02] per chunk
        r_all = const.tile([P, NJC], F32)            # exp(0.8 dst) per chunk col
        edst02_all = const.tile([P, NJC], F32)       # exp(0.2 dst)
        # qrep[j, i] = exp(-0.8 src_i), replicated across all 128 partitions
        qrep = const.tile([P, R], BF16)

        pid = nc.partition_id()
        XCH = 4                 # x chunk tiles per half of DIN
        XW = N // XCH           # 2048 cols per chunk

        # Stage-B pools open BEFORE prep so the stack allocator gives them
        # space disjoint from stage A's tiles — otherwise every adj-stream
        # DMA picks up an anti-dependency on stage A's last reads.
        adjf_pool = ctx.enter_context(tc.tile_pool(name="adjf", bufs=ADJ_BUFS))
        m_pool = ctx.enter_context(tc.tile_pool(name="m_sb", bufs=3))
        p_pool = ctx.enter_context(tc.tile_pool(name="p_sb", bufs=6))
        agg_pool = ctx.enter_context(tc.tile_pool(name="agg_ps", bufs=1, space="PSUM"))
        agg = agg_pool.tile([65, R], F32, bufs=1)

        with tc.tile_pool(name="prep", bufs=1) as prep, \
             tc.tile_pool(name="prep_ps", bufs=2, space="PSUM") as prep_ps, \
             tc.tile_pool(name="xt_ring", bufs=2) as xt_pool:
            # ---- early src path: own X columns -> diag(exp(src)) tiles ----
            xo_t = prep.tile([P, R], F32R)
            xo_b = prep.tile([P, R], F32R)
            nc.sync.dma_start(out=xo_t,
                              in_=x_d[0:P, bass.ds(pid * R, R)].bitcast(F32R))
            nc.sync.dma_start(out=xo_b,
                              in_=x_d[P:DIN, bass.ds(pid * R, R)].bitcast(F32R))
            hto_ps = prep_ps.tile([DOUT, R], F32, tag="pp", name="hto_ps")
            for hh in range(2):
                sl = slice(hh * 512, (hh + 1) * 512)
                nc.tensor.matmul(hto_ps[:, sl], lhsT=w_fr[:, 0:DOUT],
                                 rhs=xo_t[:, sl], start=True, stop=False)
                nc.tensor.matmul(hto_ps[:, sl],
                                 lhsT=w_fr[:, DOUT : 2 * DOUT],
                                 rhs=xo_b[:, sl], start=False, stop=True)
            ht_own = prep.tile([DOUT, R], F32)
            nc.scalar.copy(ht_own, hto_ps)
            s_ps = prep_ps.tile([P, SUBS], F32, tag="pp", name="s_ps")
            for q in range(SUBS):
                nc.tensor.matmul(s_ps[:, q : q + 1],
                                 lhsT=ht_own[:, q * P : (q + 1) * P], rhs=a1_f,
                                 start=True, stop=True)
            # q_i = exp(-0.8 src_i), broadcast across partitions via
            # ones128^T @ diag(q_block): column-sums of a diagonal replicate
            # the row into every partition.
            qv = prep.tile([P, SUBS], F32)
            nc.scalar.activation(qv, s_ps, mybir.ActivationFunctionType.Exp,
                                 scale=-0.8)
            ones128 = prep.tile([P, P], BF16)
            nc.vector.memset(ones128, 1.0)
            qr_ps = prep_ps.tile([P, R], F32, tag="pp", name="qr_ps")
            for b in range(SUBS):
                dgq = prep.tile([P, P], BF16, name=f"dgq{b}")
                nc.vector.tensor_scalar_mul(dgq, idbf, qv[:, b : b + 1])
                nc.tensor.matmul(qr_ps[:, b * P : (b + 1) * P],
                                 lhsT=ones128, rhs=dgq, start=True, stop=True)
            nc.vector.tensor_copy(qrep, qr_ps)

            # ---- full hT, dst exps, hext chunks ----
            # top half of X via SP (fp32, f32r matmuls); bottom half via Pool
            # (SWDGE bf16 cast) interleaved with the adj stream.
            w_bfb = const.tile([P, DOUT], BF16)
            nc.vector.tensor_copy(w_bfb, w_f[:, DOUT : 2 * DOUT])
            # xt_b: static tiles so the in-order Pool engine never blocks on a
            # ring slot (head-of-line would stall the adj stream behind it).
            xt_t, xt_b = [], []
            for k in range(XCH):
                tb = prep.tile([P, XW], BF16, tag=f"xtb{k}", name=f"xtb{k}")
                nc.gpsimd.dma_start(out=tb, in_=x_d[P:DIN, k * XW : (k + 1) * XW])
                xt_b.append(tb)
            for k in range(XCH):
                tt = xt_pool.tile([P, XW], F32R, tag="xta", name=f"xta{k}")
                nc.sync.dma_start(
                    out=tt, in_=x_d[0:P, k * XW : (k + 1) * XW].bitcast(F32R))
                xt_t.append(tt)
            ht_sb = prep.tile([DOUT, N], F32)    # h^T fp32

            for m in range(N // 512):
                k, off = m // (XW // 512), (m % (XW // 512)) * 512
                ht_ps = prep_ps.tile([DOUT, 512], F32, tag="pp", name="ht_ps")
                nc.tensor.matmul(ht_ps, lhsT=w_fr[:, 0:DOUT],
                                 rhs=xt_t[k][:, off : off + 512],
                                 start=True, stop=False)
                nc.tensor.matmul(ht_ps, lhsT=w_bfb,
                                 rhs=xt_b[k][:, off : off + 512],
                                 start=False, stop=True)
                if m % 2 == 0:
                    nc.vector.tensor_copy(ht_sb[:, m * 512 : (m + 1) * 512], ht_ps)
                else:
                    nc.scalar.copy(ht_sb[:, m * 512 : (m + 1) * 512], ht_ps)
                if m % 2 == 1:
                    b = (m - 1) // 2
                    # dst for chunks 8b..8b+7
                    d_ps = prep_ps.tile([P, 8], F32, tag="pp", name="d_ps")
                    for bb in range(8):
                        c = b * 8 + bb
                        nc.tensor.matmul(d_ps[:, bb : bb + 1],
                                         lhsT=ht_sb[:, c * P : (c + 1) * P],
                                         rhs=a2_f, start=True, stop=True)
                    nc.scalar.activation(r_all[:, b * 8 : (b + 1) * 8], d_ps,
                                         mybir.ActivationFunctionType.Exp, scale=0.8)
                    nc.scalar.activation(edst02_all[:, b * 8 : (b + 1) * 8], d_ps,
                                         mybir.ActivationFunctionType.Exp, scale=0.2)
                    # hext2: transpose 8 chunks into one PSUM tile, then copy
                    # each chunk scaled by its per-partition edst02_j.
                    hx_ps = prep_ps.tile([P, 8 * DOUT], F32, tag="pp", name="hx_ps")
                    for c in range(b * 8, (b + 1) * 8):
                        cc = c - b * 8
                        nc.tensor.transpose(
                            hx_ps[:, cc * DOUT : (cc + 1) * DOUT],
                            ht_sb[:, c * P : (c + 1) * P], idf[0:DOUT, 0:DOUT])
                    for c in range(b * 8, (b + 1) * 8):
                        cc = c - b * 8
                        dst_c = hext_all[:, c * 65 : c * 65 + DOUT]
                        src_c = hx_ps[:, cc * DOUT : (cc + 1) * DOUT]
                        sc = edst02_all[:, c : c + 1]
                        if c % 2 == 0:
                            nc.vector.tensor_scalar_mul(dst_c, src_c, sc)
                        else:
                            nc.scalar.activation(
                                dst_c, src_c,
                                mybir.ActivationFunctionType.Copy, scale=sc)
            den_ap = hext_all.rearrange("p (c s) -> p c s", s=65)[:, :, DOUT : DOUT + 1]
            nc.vector.tensor_copy(
                den_ap, edst02_all.rearrange("p (c s) -> p c s", s=1))

        # ---- stage B: main attention loop ----
        with tc.tile_pool(name="u0_ps", bufs=6, space="PSUM") as u0_pool:
            loop_cm = tc.For_i(0, rep_loop, 1) if rep_loop > 1 else nullcontext()
            with loop_cm:
                _stageB(nc, tc, adj_d, adjf_pool, u0_pool, m_pool,
                        p_pool, agg, idbf, qrep, r_all, hext_all)

        # ---- finalize ----
        with tc.tile_pool(name="fin", bufs=4) as fin, \
             tc.tile_pool(name="fin_ps", bufs=2, space="PSUM") as fin_ps:
            agg_sb = fin.tile([65, R], F32, tag="agg_sb")
            nc.vector.tensor_copy(agg_sb, agg)
            for q in range(SUBS):
                o_ps = fin_ps.tile([P, 65], F32, tag="o_ps")
                nc.tensor.matmul(o_ps, lhsT=agg_sb[:, q * P : (q + 1) * P],
                                 rhs=idf[0:65, 0:65], start=True, stop=True)
                rc = fin.tile([P, 1], F32, tag="rc")
                nc.vector.reciprocal(rc, o_ps[:, DOUT : DOUT + 1])
                hp = fin.tile([P, DOUT], F32, tag="hp")
                nc.vector.tensor_scalar_mul(hp, o_ps[:, 0:DOUT], rc)
                # elu = max(x,0) + exp(min(x,0)) - 1
                ng = fin.tile([P, DOUT], F32, tag="ng")
                nc.vector.tensor_scalar_min(ng, hp, 0.0)
                ex = fin.tile([P, DOUT], F32, tag="ex")
                nc.scalar.activation(ex, ng, mybir.ActivationFunctionType.Exp)
                ps_ = fin.tile([P, DOUT], F32, tag="ps_")
                nc.vector.tensor_scalar_max(ps_, hp, 0.0)
                ob = fin.tile([P, DOUT], F32, tag="ob")
                nc.vector.tensor_tensor(out=ob, in0=ex, in1=ps_,
                                        op=mybir.AluOpType.add)
                nc.vector.tensor_scalar_add(ob, ob, -1.0)
                nc.sync.dma_start(out=out_d[q * P : (q + 1) * P, :], in_=ob)


def _stageB(nc, tc, adj_d, adjf_pool, u0_pool, m_pool, p_pool,
            agg, idbf, qrep, r_all, hext_all):
    for jt in range(NJT):
        adjf = []
        for s in range(SUBS):
            # uint8 adj (packed on host) via Pool SWDGE, cast to fp8 0/1.0:
            # 4x less HBM read than int32, half the SBUF write of bf16.  The
            # fp8 lhsT x bf16 rhs matmul is exact (0/1 weights).
            t = adjf_pool.tile([P, JT_W], BF16, tag="adjf")
            if ABLATE_DMA:
                nc.gpsimd.dma_start(
                    out=t[:, 0:128],
                    in_=adj_d[s * P : (s + 1) * P, jt * JT_W : jt * JT_W + 128],
                )
            else:
                nc.gpsimd.dma_start(
                    out=t,
                    in_=adj_d[s * P : (s + 1) * P, jt * JT_W : (jt + 1) * JT_W],
                )
            adjf.append(t)
        for kk in range(JCPJT):
            jc = jt * JCPJT + kk
            # m[j, i] = max(r_j, q_i): value tensor as an outer max — the
            # per-i scale esrc_i cancels in the finalize ratio, so only the
            # MASK needs the PE (plain bf16 transposes, no diag branches).
            m = m_pool.tile([P, 1024], BF16, tag="m")
            nc.vector.tensor_scalar_max(m, qrep, r_all[:, jc : jc + 1])
            for h in range(2):          # i halves of 512
                u0 = u0_pool.tile([P, 512], BF16, tag="u0")
                for qq in range(4):
                    q = 4 * h + qq
                    lt = adjf[q][:, kk * P : (kk + 1) * P]
                    nc.tensor.transpose(u0[:, qq * P : (qq + 1) * P], lt, idbf)
                p = p_pool.tile([P, 512], BF16, tag="p")
                nc.vector.tensor_tensor(
                    out=p, in0=u0, in1=m[:, h * 512 : (h + 1) * 512],
                    op=mybir.AluOpType.mult)
                nc.tensor.matmul(agg[:, h * 512 : (h + 1) * 512],
                                 lhsT=hext_all[:, jc * 65 : jc * 65 + 65], rhs=p,
                                 start=(jc == 0), stop=(jc == NJC - 1))


def kernel(**inputs) -> np.ndarray:
    xt = np.ascontiguousarray(np.asarray(inputs["input"], np.float32)[0].T)
    adj = np.ascontiguousarray(np.asarray(inputs["adj"]).astype(np.uint8))
    w = np.ascontiguousarray(np.asarray(inputs["w"], np.float32))
    a = np.ascontiguousarray(np.asarray(inputs["a"], np.float32).reshape(2 * DOUT, 1))

    nc = build_module()
    in_maps = []
    for c in range(NCORES):
        in_maps.append({
            "adjslab": adj[c * R : (c + 1) * R, :],
            "xt": xt,
            "w": w,
            "av": a,
        })
    res = bass_utils.run_bass_kernel_spmd(nc, in_maps, core_ids=list(range(NCORES)))
    out = np.concatenate([res.results[c]["out"] for c in range(NCORES)], axis=0)
    return out.astype(np.float32)


if __name__ == "__main__":
    rng = np.random.default_rng(0)
    ins = {
        "input": rng.standard_normal((1, N, DIN)).astype(np.float32),
        "adj": rng.integers(0, 2, size=(N, N)).astype(np.int32),
        "w": rng.standard_normal((DIN, DOUT)).astype(np.float32) * 0.1,
        "a": rng.standard_normal((2 * DOUT, 1)).astype(np.float32) * 0.1,
    }
    o = kernel(**ins)
    print("kernel out", o.shape, o.dtype)



# revision 28
# speedup vs baseline: 1.1973x; 1.1973x over previous
"""GAT head (DGAT) Trainium2 kernel: 8-core row-sharded masked-softmax attention.

Math (per reference):
  h = X @ W                       [N, 64]
  s_ij = src_i + dst_j, src = h@a[:64], dst = h@a[64:]
  att = softmax(where(adj>0, leaky_relu(s, 0.2), -inf), axis=1)
  out = elu(att @ h)

Key factorization (outer-max): with E = exp(s) = exp(src_i)*exp(dst_j),
exp(leaky_relu(s)) = max(E, E^0.2), the masked unnormalized attention
  p[j,i] = adjT[j,i] * max(esrc_i*edst_j, esrc02_i*edst02_j)
         = adjT[j,i] * esrc_i * edst02_j * max(r_j, q_i)
with r_j = exp(0.8*dst_j), q_i = exp(-0.8*src_i).  The per-j factor
edst02_j folds into the precomputed hext (hext2[j,f] = hext[j,f] *
edst02_j, denominator row 64 = edst02_j); the per-i factor esrc_i
passes straight through the j-contraction and CANCELS in the finalize
ratio out = elu(num_i/den_i) — no compensation needed.  So stage B is:
  DVE: m = max(r_j, q_i)      (tensor_scalar_max on a precomputed
       partition-replicated q row; all-SBUF bf16, chunk-invariant q)
  PE:  u0 = adjT              (plain bf16 transposes, bf16 PSUM out —
       no diag branches, half the old matmul work)
  DVE: p = u0 * m             (tensor_tensor mult, bf16 PSUM in0)
  PE:  agg += hext2^T @ p
ACT does nothing in stage B; the old u1/u2 diag-ride + scaled-copy +
max pipeline (PE 1.28us, ACT 1.1us, DVE 1.3us per chunk) collapses to
PE ~0.85us, DVE ~1.0us per chunk.

DMA split: adj is packed to uint8 on the host and streamed via Pool
SWDGE with a cast to bf16 {0,1.0};
SP (HWDGE) carries X fp32, consumed via float32r matmuls (1 cyc/col).
A dedicated early-src path (dynamic-offset DMA of this core's X columns)
gets the diag(exp(src)) tiles ready ~8us in so stage B starts early.

Aggregation: agg[65, i] += [h | 1]^T @ p  (row 64 = softmax denominator),
finalize: transpose agg, scale by 1/denom, ELU, DMA out.
"""
import os
import sys
import numpy as np

sys.path.insert(0, "/opt/trn_rl_repo")

import concourse.bass as bass
import concourse.bacc as bacc
import concourse.tile as tile
from concourse import mybir
from concourse.masks import make_identity
from concourse import bass_utils

P = 128
N = 8192
DIN = 256
DOUT = 64
NCORES = 8
R = N // NCORES          # rows per core
JT_W = 2048              # j supertile width
NJT = N // JT_W
JCPJT = JT_W // P
NJC = N // P             # 64 j-chunks
SUBS = R // P            # 8 row sub-blocks per core
F32 = mybir.dt.float32
F32R = mybir.dt.float32r
BF16 = mybir.dt.bfloat16
I32 = mybir.dt.int32
U8 = mybir.dt.uint8
FP8 = mybir.dt.float8e4

_cached = {}
ADJ_BUFS = 24
UBUFS = 3
ABLATE_DMA = bool(int(os.environ.get("ABLATE_DMA", "0")))   # dev-only timing probe
ABLATE_DVE = bool(int(os.environ.get("ABLATE_DVE", "0")))   # dev-only timing probe
ABLATE_ACT = bool(int(os.environ.get("ABLATE_ACT", "0")))   # dev-only timing probe


def build_module(rep=1, rep_loop=1):
    key = ("nc", rep, rep_loop)
    if key in _cached:
        return _cached[key]
    nc = bacc.Bacc("TRN2", target_bir_lowering=False, debug=False, num_devices=NCORES)

    adj_d = nc.dram_tensor("adjslab", [R, N], U8, kind="ExternalInput").ap()
    x_d = nc.dram_tensor("xt", [DIN, N], F32, kind="ExternalInput").ap()
    w_d = nc.dram_tensor("w", [DIN, DOUT], F32, kind="ExternalInput").ap()
    a_d = nc.dram_tensor("av", [2 * DOUT, 1], F32, kind="ExternalInput").ap()
    out_d = nc.dram_tensor("out", [R, DOUT], F32, kind="ExternalOutput").ap()

    with tile.TileContext(nc) as tc:
        for _ in range(rep):
            _build(nc, tc, adj_d, x_d, w_d, a_d, out_d, rep_loop)

    nc.compile()
    _cached[key] = nc
    return nc


def _build(nc, tc, adj_d, x_d, w_d, a_d, out_d, rep_loop=1):
    from contextlib import ExitStack, nullcontext

    with ExitStack() as ctx:
        const = ctx.enter_context(tc.tile_pool(name="const", bufs=1))

        # ---- constants ----
        idbf = const.tile([P, P], BF16)
        make_identity(nc, idbf)
        idf = const.tile([P, P], F32)
        make_identity(nc, idf)

        w_f = const.tile([P, 2 * DOUT], F32)   # [din_blk, (a|b) x 64]
        nc.sync.dma_start(out=w_f[:, 0:DOUT], in_=w_d[0:P, :])
        nc.sync.dma_start(out=w_f[:, DOUT : 2 * DOUT], in_=w_d[P:DIN, :])
        w_fr = const.tile([P, 2 * DOUT], F32R)  # f32r twin for PE use
        nc.sync.dma_start(out=w_fr[:, 0:DOUT], in_=w_d[0:P, :].bitcast(F32R))
        nc.sync.dma_start(out=w_fr[:, DOUT : 2 * DOUT], in_=w_d[P:DIN, :].bitcast(F32R))
        a1_f = const.tile([DOUT, 1], F32)
        a2_f = const.tile([DOUT, 1], F32)
        nc.sync.dma_start(out=a1_f, in_=a_d[0:DOUT, :])
        nc.sync.dma_start(out=a2_f, in_=a_d[DOUT : 2 * DOUT, :])

        # persistent per-core data
        hext_all = const.tile([P, NJC * 65], BF16)   # hext2: [j%P, 64 h*edst02 cols + edst02] per chunk
        r_all = const.tile([P, NJC], F32)            # exp(0.8 dst) per chunk col
        edst02_all = const.tile([P, NJC], F32)       # exp(0.2 dst)
        # qrep[j, i] = exp(-0.8 src_i), replicated across all 128 partitions
        qrep = const.tile([P, R], BF16)

        pid = nc.partition_id()
        XCH = 4                 # x chunk tiles per half of DIN
        XW = N // XCH           # 2048 cols per chunk

        # Stage-B pools open BEFORE prep so the stack allocator gives them
        # space disjoint from stage A's tiles — otherwise every adj-stream
        # DMA picks up an anti-dependency on stage A's last reads.
        adjf_pool = ctx.enter_context(tc.tile_pool(name="adjf", bufs=ADJ_BUFS))
        m_pool = ctx.enter_context(tc.tile_pool(name="m_sb", bufs=3))
        p_pool = ctx.enter_context(tc.tile_pool(name="p_sb", bufs=6))
        agg_pool = ctx.enter_context(tc.tile_pool(name="agg_ps", bufs=1, space="PSUM"))
        agg = agg_pool.tile([65, R], F32, bufs=1)

        with tc.tile_pool(name="prep", bufs=1) as prep, \
             tc.tile_pool(name="prep_ps", bufs=2, space="PSUM") as prep_ps, \
             tc.tile_pool(name="xt_ring", bufs=2) as xt_pool:
            # ---- early src path: own X columns -> diag(exp(src)) tiles ----
            xo_t = prep.tile([P, R], F32R)
            xo_b = prep.tile([P, R], F32R)
            nc.sync.dma_start(out=xo_t,
                              in_=x_d[0:P, bass.ds(pid * R, R)].bitcast(F32R))
            nc.sync.dma_start(out=xo_b,
                              in_=x_d[P:DIN, bass.ds(pid * R, R)].bitcast(F32R))
            hto_ps = prep_ps.tile([DOUT, R], F32, tag="pp", name="hto_ps")
            for hh in range(2):
                sl = slice(hh * 512, (hh + 1) * 512)
                nc.tensor.matmul(hto_ps[:, sl], lhsT=w_fr[:, 0:DOUT],
                                 rhs=xo_t[:, sl], start=True, stop=False)
                nc.tensor.matmul(hto_ps[:, sl],
                                 lhsT=w_fr[:, DOUT : 2 * DOUT],
                                 rhs=xo_b[:, sl], start=False, stop=True)
            ht_own = prep.tile([DOUT, R], F32)
            nc.scalar.copy(ht_own, hto_ps)
            s_ps = prep_ps.tile([P, SUBS], F32, tag="pp", name="s_ps")
            for q in range(SUBS):
                nc.tensor.matmul(s_ps[:, q : q + 1],
                                 lhsT=ht_own[:, q * P : (q + 1) * P], rhs=a1_f,
                                 start=True, stop=True)
            # q_i = exp(-0.8 src_i), broadcast across partitions via
            # ones128^T @ diag(q_block): column-sums of a diagonal replicate
            # the row into every partition.
            qv = prep.tile([P, SUBS], F32)
            nc.scalar.activation(qv, s_ps, mybir.ActivationFunctionType.Exp,
                                 scale=-0.8)
            ones128 = prep.tile([P, P], BF16)
            nc.vector.memset(ones128, 1.0)
            qr_ps = prep_ps.tile([P, R], F32, tag="pp", name="qr_ps")
            for b in range(SUBS):
                dgq = prep.tile([P, P], BF16, name=f"dgq{b}")
                nc.vector.tensor_scalar_mul(dgq, idbf, qv[:, b : b + 1])
                nc.tensor.matmul(qr_ps[:, b * P : (b + 1) * P],
                                 lhsT=ones128, rhs=dgq, start=True, stop=True)
            nc.vector.tensor_copy(qrep, qr_ps)

            # ---- full hT, dst exps, hext chunks ----
            # top half of X via SP (fp32, f32r matmuls); bottom half via Pool
            # (SWDGE bf16 cast) interleaved with the adj stream.
            w_bfb = const.tile([P, DOUT], BF16)
            nc.vector.tensor_copy(w_bfb, w_f[:, DOUT : 2 * DOUT])
            # xt_b: static tiles so the in-order Pool engine never blocks on a
            # ring slot (head-of-line would stall the adj stream behind it).
            xt_t, xt_b = [], []
            for k in range(XCH):
                tb = prep.tile([P, XW], BF16, tag=f"xtb{k}", name=f"xtb{k}")
                nc.gpsimd.dma_start(out=tb, in_=x_d[P:DIN, k * XW : (k + 1) * XW])
                xt_b.append(tb)
            for k in range(XCH):
                tt = xt_pool.tile([P, XW], F32R, tag="xta", name=f"xta{k}")
                nc.sync.dma_start(
                    out=tt, in_=x_d[0:P, k * XW : (k + 1) * XW].bitcast(F32R))
                xt_t.append(tt)
            ht_sb = prep.tile([DOUT, N], F32)    # h^T fp32

            for m in range(N // 512):
                k, off = m // (XW // 512), (m % (XW // 512)) * 512
                ht_ps = prep_ps.tile([DOUT, 512], F32, tag="pp", name="ht_ps")
                nc.tensor.matmul(ht_ps, lhsT=w_fr[:, 0:DOUT],
                                 rhs=xt_t[k][:, off : off + 512],
                                 start=True, stop=False)
                nc.tensor.matmul(ht_ps, lhsT=w_bfb,
                                 rhs=xt_b[k][:, off : off + 512],
                                 start=False, stop=True)
                if m % 2 == 0:
                    nc.vector.tensor_copy(ht_sb[:, m * 512 : (m + 1) * 512], ht_ps)
                else:
                    nc.scalar.copy(ht_sb[:, m * 512 : (m + 1) * 512], ht_ps)
                if m % 2 == 1:
                    b = (m - 1) // 2
                    # dst for chunks 8b..8b+7
                    d_ps = prep_ps.tile([P, 8], F32, tag="pp", name="d_ps")
                    for bb in range(8):
                        c = b * 8 + bb
                        nc.tensor.matmul(d_ps[:, bb : bb + 1],
                                         lhsT=ht_sb[:, c * P : (c + 1) * P],
                                         rhs=a2_f, start=True, stop=True)
                    nc.scalar.activation(r_all[:, b * 8 : (b + 1) * 8], d_ps,
                                         mybir.ActivationFunctionType.Exp, scale=0.8)
                    nc.scalar.activation(edst02_all[:, b * 8 : (b + 1) * 8], d_ps,
                                         mybir.ActivationFunctionType.Exp, scale=0.2)
                    # hext2: transpose 8 chunks into one PSUM tile, then copy
                    # each chunk scaled by its per-partition edst02_j.
                    hx_ps = prep_ps.tile([P, 8 * DOUT], F32, tag="pp", name="hx_ps")
                    for c in range(b * 8, (b + 1) * 8):
                        cc = c - b * 8
                        nc.tensor.transpose(
                            hx_ps[:, cc * DOUT : (cc + 1) * DOUT],
                            ht_sb[:, c * P : (c + 1) * P], idf[0:DOUT, 0:DOUT])
                    for c in range(b * 8, (b + 1) * 8):
                        cc = c - b * 8
                        dst_c = hext_all[:, c * 65 : c * 65 + DOUT]
                        src_c = hx_ps[:, cc * DOUT : (cc + 1) * DOUT]
                        sc = edst02_all[:, c : c + 1]
                        if c % 2 == 0:
                            nc.vector.tensor_scalar_mul(dst_c, src_c, sc)
                        else:
                            nc.scalar.activation(
                                dst_c, src_c,
                                mybir.ActivationFunctionType.Copy, scale=sc)
            den_ap = hext_all.rearrange("p (c s) -> p c s", s=65)[:, :, DOUT : DOUT + 1]
            nc.vector.tensor_copy(
                den_ap, edst02_all.rearrange("p (c s) -> p c s", s=1))

        # ---- stage B: main attention loop ----
        with tc.tile_pool(name="u0_ps", bufs=6, space="PSUM") as u0_pool:
            loop_cm = tc.For_i(0, rep_loop, 1) if rep_loop > 1 else nullcontext()
            with loop_cm:
                _stageB(nc, tc, adj_d, adjf_pool, u0_pool, m_pool,
                        p_pool, agg, idbf, qrep, r_all, hext_all)

        # ---- finalize ----
        with tc.tile_pool(name="fin", bufs=4) as fin, \
             tc.tile_pool(name="fin_ps", bufs=2, space="PSUM") as fin_ps:
            agg_sb = fin.tile([65, R], F32, tag="agg_sb")
            nc.vector.tensor_copy(agg_sb, agg)
            for q in range(SUBS):
                o_ps = fin_ps.tile([P, 65], F32, tag="o_ps")
                nc.tensor.matmul(o_ps, lhsT=agg_sb[:, q * P : (q + 1) * P],
                                 rhs=idf[0:65, 0:65], start=True, stop=True)
                rc = fin.tile([P, 1], F32, tag="rc")
                nc.vector.reciprocal(rc, o_ps[:, DOUT : DOUT + 1])
                hp = fin.tile([P, DOUT], F32, tag="hp")
                nc.vector.tensor_scalar_mul(hp, o_ps[:, 0:DOUT], rc)
                # elu = max(x,0) + exp(min(x,0)) - 1
                ng = fin.tile([P, DOUT], F32, tag="ng")
                nc.vector.tensor_scalar_min(ng, hp, 0.0)
                ex = fin.tile([P, DOUT], F32, tag="ex")
                nc.scalar.activation(ex, ng, mybir.ActivationFunctionType.Exp)
                ps_ = fin.tile([P, DOUT], F32, tag="ps_")
                nc.vector.tensor_scalar_max(ps_, hp, 0.0)
                ob = fin.tile([P, DOUT], F32, tag="ob")
                nc.vector.tensor_tensor(out=ob, in0=ex, in1=ps_,
                                        op=mybir.AluOpType.add)
                nc.vector.tensor_scalar_add(ob, ob, -1.0)
                nc.sync.dma_start(out=out_d[q * P : (q + 1) * P, :], in_=ob)


def _stageB(nc, tc, adj_d, adjf_pool, u0_pool, m_pool, p_pool,
            agg, idbf, qrep, r_all, hext_all):
    for jt in range(NJT):
        adjf = []
        for s in range(SUBS):
            # uint8 adj (packed on host) via Pool SWDGE, cast to fp8 0/1.0:
            # 4x less HBM read than int32, half the SBUF write of bf16.  The
            # fp8 lhsT x bf16 rhs matmul is exact (0/1 weights).
            t = adjf_pool.tile([P, JT_W], BF16, tag="adjf")
            if ABLATE_DMA:
                nc.gpsimd.dma_start(
                    out=t[:, 0:128],
                    in_=adj_d[s * P : (s + 1) * P, jt * JT_W : jt * JT_W + 128],
                )
            else:
                nc.gpsimd.dma_start(
                    out=t,
                    in_=adj_d[s * P : (s + 1) * P, jt * JT_W : (jt + 1) * JT_W],
                )
            adjf.append(t)
        for kk in range(JCPJT):
            jc = jt * JCPJT + kk
            # m[j, i] = max(r_j, q_i): value tensor as an outer max — the
            # per-i scale esrc_i cancels in the finalize ratio, so only the
            # MASK needs the PE (plain bf16 transposes, no diag branches).
            m = m_pool.tile([P, 1024], BF16, tag="m")
            nc.vector.tensor_scalar_max(m, qrep, r_all[:, jc : jc + 1])
            for h in range(2):          # i halves of 512
                u0 = u0_pool.tile([P, 512], BF16, tag="u0")
                for qq in range(4):
                    q = 4 * h + qq
                    lt = adjf[q][:, kk * P : (kk + 1) * P]
                    nc.tensor.transpose(u0[:, qq * P : (qq + 1) * P], lt, idbf)
                p = p_pool.tile([P, 512], BF16, tag="p")
                nc.vector.tensor_tensor(
                    out=p, in0=u0, in1=m[:, h * 512 : (h + 1) * 512],
                    op=mybir.AluOpType.mult)
                nc.tensor.matmul(agg[:, h * 512 : (h + 1) * 512],
                                 lhsT=hext_all[:, jc * 65 : jc * 65 + 65], rhs=p,
                                 start=(jc == 0), stop=(jc == NJC - 1))


def kernel(**inputs) -> np.ndarray:
    xt = np.ascontiguousarray(np.asarray(inputs["input"], np.float32)[0].T)
    adj = np.ascontiguousarray(np.asarray(inputs["adj"]).astype(np.uint8))
    w = np.ascontiguousarray(np.asarray(inputs["w"], np.float32))
    a = np.ascontiguousarray(np.asarray(inputs["a"], np.float32).reshape(2 * DOUT, 1))

    nc = build_module()
    in_maps = []
    for c in range(NCORES):
        in_maps.append({
            "adjslab": adj[c * R : (c + 1) * R, :],
            "xt": xt,
            "w": w,
            "av": a,
        })
    res = bass_utils.run_bass_kernel_spmd(nc, in_maps, core_ids=list(range(NCORES)))
    out = np.concatenate([res.results[c]["out"] for c in range(NCORES)], axis=0)
    return out.astype(np.float32)


if __name__ == "__main__":
    rng = np.random.default_rng(0)
    ins = {
        "input": rng.standard_normal((1, N, DIN)).astype(np.float32),
        "adj": rng.integers(0, 2, size=(N, N)).astype(np.int32),
        "w": rng.standard_normal((DIN, DOUT)).astype(np.float32) * 0.1,
        "a": rng.standard_normal((2 * DOUT, 1)).astype(np.float32) * 0.1,
    }
    o = kernel(**ins)
    print("kernel out", o.shape, o.dtype)

